# revision 1
# baseline (speedup 1.0000x reference)
"""DeepseekV3 top-k router kernel for Trainium2 (Bass/Tile), 8-core SPMD.

Reference computation (per token, 256 experts):
  s    = sigmoid(logits)            computed as 1/(1+exp(-x)) with the DVE's
                                    bit-exact reciprocal -- bitwise identical
                                    to jax-on-neuron's logistic lowering
  s4c  = s + correction_bias
  group scores = sum of top-2 of s4c within each of 8 groups of 32
  top-4 groups -> mask -> masked s4c
  top-8 of masked s4c -> (indices, values)   [DVE Max/MaxIndex: jax tie rules]
  weights = s at those indices, normalized to sum 2.5

Sharding: data-parallel on the token dim across 8 cores (16384 tokens each);
the 256-entry correction bias is replicated. Layout: one token per SBUF
partition, its 256 expert scores along the free dim; 128 tokens per tile,
8 tiles per "supertile" so the narrow [P,8]-ish stages run batched.

Engine split: ACT does exp and the +1; the Pool (gpsimd) engine does the
bias add; DVE does the reciprocal, group maxes (the second max via a fused
custom DVE op), masking, and the Max8/FindIndex8/MatchReplace extraction.
"""
import numpy as np

import concourse.bass as bass
import concourse.bacc as bacc
import concourse.mybir as mybir
from concourse.tile import TileContext
from concourse.bass_utils import run_bass_kernel_spmd

F32 = mybir.dt.float32
U32 = mybir.dt.uint32

T_FULL = 131072
E = 256
N_CORES = 8
T_CORE = T_FULL // N_CORES      # 16384
P = 128                         # tokens per tile (one per partition)
N_TILES = T_CORE // P           # 128
B = 8                           # tiles per supertile
N_SUPER = N_TILES // B
G = 8                           # expert groups
EG = E // G                     # experts per group
BIG = 1.0e30

LAST_EXEC_NS = None
LAST_RESULTS = None

_EQNEG = None


def _get_eqneg():
    """Fused custom DVE op: out = (in0 == in1) ? -FLT_MAX : in0 (one pass,
    replaces a tensor_tensor(is_equal) + scalar_tensor_tensor pair)."""
    global _EQNEG
    if _EQNEG is None:
        from concourse.dve_ops import (DveOp, OPS, get_dve_sub_opcode,
                                       has_src1)
        from concourse.dve_spec import Spec, Src0, Src1, MaxNeg, select, eq, lower
        from concourse.dve_uop import DveOpSpec
        import concourse.dve_ops as dve_ops_mod

        spec = Spec(
            body=select(eq(Src0, Src1), MaxNeg, Src0),
            reference=lambda in0, in1, s0, s1, imm2: np.where(
                in0 == in1, np.float32(-3.4028234663852886e38), in0
            ).astype(np.float32),
        )
        op = DveOp("RTR_EQNEG", spec, subdim=False, uops_sha={})
        OPS.append(op)
        dve_ops_mod.CUSTOM_DVE_SPECS[op.name] = op.spec
        dve_ops_mod._SUB_OPCODE_FOR_NAME[op.name] = (
            dve_ops_mod._CUSTOM_DVE_ROW_BASE + len(OPS) - 1)
        assert dve_ops_mod._SUB_OPCODE_FOR_NAME[op.name] < 0x20
        for ver in ("v3", "v4"):
            tmp = DveOpSpec(name=op.name, opcode=get_dve_sub_opcode(op.name),
                            uops=lower(spec, ver=ver), rd1_en=has_src1(spec))
            op.uops_sha[ver] = tmp.sha(ver)
        _EQNEG = op
    return _EQNEG


def _build(nc: bass.Bass):
    x_d = nc.dram_tensor("logits", [T_CORE, E], F32, kind="ExternalInput")
    b_d = nc.dram_tensor("bias", [1, E], F32, kind="ExternalInput")
    idx_d = nc.dram_tensor("idx_out", [T_CORE, 8], U32, kind="ExternalOutput")
    w_d = nc.dram_tensor("w_out", [T_CORE, 8], F32, kind="ExternalOutput")

    AX = mybir.AxisListType.X
    OP = mybir.AluOpType
    ACTF = mybir.ActivationFunctionType
    eqneg = _get_eqneg()

    with TileContext(nc) as tc:
        with tc.tile_pool(name="const", bufs=1) as cpool, \
             tc.tile_pool(name="io", bufs=2) as iopool, \
             tc.tile_pool(name="wide", bufs=4) as wpool, \
             tc.tile_pool(name="keep", bufs=2) as kpool, \
             tc.tile_pool(name="slot", bufs=2) as slpool, \
             tc.tile_pool(name="small", bufs=4) as spool:

            biasb = cpool.tile([P, E], F32)
            nc.gpsimd.dma_start(out=biasb[:], in_=b_d[:, :].to_broadcast((P, E)))

            for sp_i in range(N_SUPER):
                m1s = slpool.tile([P, B * G], F32, tag="m1s")
                m2s = slpool.tile([P, B * G], F32, tag="m2s")
                gss = slpool.tile([P, B * G], F32, tag="gss")
                gts = slpool.tile([P, B * G * G], F32, tag="gts")
                ranks = slpool.tile([P, B * G], F32, tag="ranks")
                v8s = slpool.tile([P, B * 8], F32, tag="v8s")
                i8s = slpool.tile([P, B * 8], U32, tag="i8s")
                sv8s = slpool.tile([P, B * 8], F32, tag="sv8s")
                si8s = slpool.tile([P, B * 8], U32, tag="si8s")
                dens = slpool.tile([P, B], F32, tag="dens")
                rdens = slpool.tile([P, B], F32, tag="rdens")
                eqms = slpool.tile([P, B * 64], F32, tag="eqms")
                wms = slpool.tile([P, B * 64], F32, tag="wms")
                w8s = slpool.tile([P, B * 8], F32, tag="w8s")
                wouts = slpool.tile([P, B * 8], F32, tag="wouts")

                # One 1MB load per supertile. Token mapping inside the
                # supertile: partition p, tile b <-> token sp_i*1024 + 8p + b
                # (the output DMAs write the same mapping, so the DRAM
                # result is in natural token order).
                srow = sp_i * B * P
                Ls = iopool.tile([P, B * E], F32, tag="L")
                nc.sync.dma_start(
                    out=Ls[:],
                    in_=x_d[srow:srow + B * P, :].rearrange(
                        "(p x) e -> p (x e)", p=P))

                s_keep = []
                s4c_keep = []
                # ---- phase A: per tile, sigmoid / bias / group top-2 ----
                for b in range(B):
                    L = Ls[:, b * E:(b + 1) * E]

                    e = wpool.tile([P, E], F32, tag="e")
                    nc.scalar.activation(e[:], L, ACTF.Exp, scale=-1.0)
                    u = wpool.tile([P, E], F32, tag="u")
                    nc.scalar.activation(u[:], e[:], ACTF.Copy, bias=1.0)
                    s = kpool.tile([P, E], F32, tag=f"s{b}")
                    nc.vector.reciprocal(s[:], u[:])
                    s_keep.append(s)

                    s4c = kpool.tile([P, E], F32, tag=f"s4c{b}")
                    nc.gpsimd.tensor_tensor(s4c[:], s[:], biasb[:], op=OP.add)
                    s4c_keep.append(s4c)
                    s4c_g = s4c[:].rearrange("p (g e) -> p g e", g=G)

                    m1v = m1s[:, b * G:(b + 1) * G]
                    nc.vector.tensor_reduce(m1v, s4c_g, axis=AX, op=OP.max)
                    t2 = wpool.tile([P, E], F32, tag="t2")
                    nc.vector._custom_dve(
                        eqneg,
                        out=t2[:].rearrange("p (g e) -> p g e", g=G),
                        in0=s4c_g,
                        in1=m1v.unsqueeze(2).broadcast_to([P, G, EG]))
                    nc.vector.tensor_reduce(
                        m2s[:, b * G:(b + 1) * G],
                        t2[:].rearrange("p (g e) -> p g e", g=G),
                        axis=AX, op=OP.max)

                # ---- phase B: batched group ranking ----
                nc.vector.tensor_add(gss[:], m1s[:], m2s[:])
                gs3 = gss[:].rearrange("p (b g) -> p b g", b=B)
                nc.vector.tensor_tensor(
                    gts[:].rearrange("p (b i j) -> p b i j", b=B, i=G),
                    gs3.unsqueeze(2).broadcast_to([P, B, G, G]),
                    gs3.unsqueeze(3).broadcast_to([P, B, G, G]),
                    op=OP.is_gt)
                nc.vector.tensor_reduce(
                    ranks[:], gts[:].rearrange("p (b i j) -> p b i j", b=B, i=G),
                    axis=AX, op=OP.add)

                # ---- phase C: per tile top-8 extraction ----
                for b in range(B):
                    s = s_keep[b]
                    s4c = s4c_keep[b]
                    s4c_g = s4c[:].rearrange("p (g e) -> p g e", g=G)

                    masked = wpool.tile([P, E], F32, tag="masked")
                    rankv = ranks[:, b * G:(b + 1) * G]
                    nc.vector.scalar_tensor_tensor(
                        masked[:].rearrange("p (g e) -> p g e", g=G),
                        rankv.unsqueeze(2).broadcast_to([P, G, EG]), 4.0,
                        s4c_g, op0=OP.is_lt, op1=OP.mult)

                    v8 = v8s[:, b * 8:(b + 1) * 8]
                    nc.vector.max(out=v8, in_=masked[:])
                    nc.vector.max_index(i8s[:, b * 8:(b + 1) * 8], v8, masked[:])

                    marked = wpool.tile([P, E], F32, tag="marked")
                    nc.vector.match_replace(out=marked[:], in_to_replace=v8,
                                            in_values=masked[:], imm_value=BIG)
                    s_sel = wpool.tile([P, E], F32, tag="s_sel")
                    nc.vector.scalar_tensor_tensor(
                        s_sel[:], marked[:], BIG, s[:],
                        op0=OP.is_equal, op1=OP.mult,
                        accum_out=dens[:, b:b + 1])

                    sv8 = sv8s[:, b * 8:(b + 1) * 8]
                    nc.vector.max(out=sv8, in_=s_sel[:])
                    nc.vector.max_index(si8s[:, b * 8:(b + 1) * 8], sv8,
                                        s_sel[:])

                # ---- phase D: batched realign + normalize + store ----
                i8s3 = i8s[:].rearrange("p (b i) -> p b i", b=B)
                si8s3 = si8s[:].rearrange("p (b k) -> p b k", b=B)
                nc.vector.tensor_tensor(
                    eqms[:].rearrange("p (b i k) -> p b i k", b=B, i=8),
                    i8s3.unsqueeze(3).broadcast_to([P, B, 8, 8]),
                    si8s3.unsqueeze(2).broadcast_to([P, B, 8, 8]),
                    op=OP.is_equal)
                sv3 = sv8s[:].rearrange("p (b k) -> p b k", b=B)
                nc.vector.tensor_tensor(
                    wms[:].rearrange("p (b i k) -> p b i k", b=B, i=8),
                    eqms[:].rearrange("p (b i k) -> p b i k", b=B, i=8),
                    sv3.unsqueeze(2).broadcast_to([P, B, 8, 8]),
                    op=OP.mult)
                nc.vector.tensor_reduce(
                    w8s[:], wms[:].rearrange("p (b i k) -> p b i k", b=B, i=8),
                    axis=AX, op=OP.add)
                nc.vector.reciprocal(rdens[:], dens[:])
                rd3 = rdens[:].rearrange("p (b o) -> p b o", b=B)
                nc.vector.scalar_tensor_tensor(
                    wouts[:].rearrange("p (b i) -> p b i", b=B),
                    w8s[:].rearrange("p (b i) -> p b i", b=B), 2.5,
                    rd3.broadcast_to([P, B, 8]),
                    op0=OP.mult, op1=OP.mult)

                nc.sync.dma_start(
                    out=idx_d[srow:srow + B * P, :].rearrange(
                        "(p x) e -> p (x e)", p=P),
                    in_=i8s[:])
                nc.sync.dma_start(
                    out=w_d[srow:srow + B * P, :].rearrange(
                        "(p x) e -> p (x e)", p=P),
                    in_=wouts[:])
    return nc


_COMPILED_NC = None


def _get_nc():
    global _COMPILED_NC
    if _COMPILED_NC is None:
        nc = bacc.Bacc(None, target_bir_lowering=False, debug=False)
        _build(nc)
        nc.finalize()
        _COMPILED_NC = nc
    return _COMPILED_NC


def kernel(router_logits: np.ndarray, correction_bias: np.ndarray,
           trace: bool = False):
    global LAST_EXEC_NS, LAST_RESULTS
    x = np.ascontiguousarray(np.asarray(router_logits), dtype=np.float32)
    b = np.ascontiguousarray(np.asarray(correction_bias),
                             dtype=np.float32).reshape(1, E)
    assert x.shape == (T_FULL, E), x.shape

    nc = _get_nc()
    in_maps = [{"logits": x[c * T_CORE:(c + 1) * T_CORE], "bias": b}
               for c in range(N_CORES)]
    res = run_bass_kernel_spmd(nc, in_maps, core_ids=list(range(N_CORES)),
                               trace=trace)
    LAST_EXEC_NS = res.exec_time_ns
    LAST_RESULTS = res

    idx = np.concatenate([r["idx_out"] for r in res.results], axis=0)
    w = np.concatenate([r["w_out"] for r in res.results], axis=0)
    return idx.view(np.int32), w.astype(np.float32, copy=False)



# revision 3
# speedup vs baseline: 1.7521x; 1.7521x over previous
"""DeepseekV3 top-k router kernel for Trainium2 (Bass/Tile), 8-core SPMD.

Reference computation (per token, 256 experts):
  s    = sigmoid(logits)           ACT-engine Sigmoid table (~1e-6 max abs err)
  s4c  = s + correction_bias       gpsimd add
  group scores = sum of top-2 of s4c within each of 8 groups of 32
  top-4 groups -> mask -> masked s4c
  top-8 of masked s4c -> (indices, values)   [DVE Max8/FindIndex8]
  weights = s at those indices, normalized to sum 2.5

Weight extraction avoids a second FindIndex8 + index matching via a packed
"carrier": z = K*s4c + bias at the 8 selected positions (0 elsewhere), with
K = 2^14.  Max8(z) returns the selected entries in (approximately) the same
order as Max8(masked); then s_at_selected = (1+K)*v8 - z8 elementwise.  The
packing costs ~1.5e-3 relative error on the weights (fp32 low-bit truncation
of the bias payload + rare near-tie order swaps), well under the 2e-2 gate;
indices are unaffected.

Sharding: data-parallel on the token dim across 8 cores (16384 tokens each);
the 256-entry bias is replicated (host pre-tiles it 8x to [1, 2048]).
Layout: one token per SBUF partition, its 256 expert scores along the free
dim; 128 tokens per tile, 8 tiles per supertile [128, 2048] so the wide
stages run as single instructions.

Engine split: ACT does the sigmoid; gpsimd does the bias add and the
group-mask application; DVE does the group maxes (second max via a fused
custom eqneg op), rank compare, Max8/FindIndex8, and the carrier op.  The
per-supertile loop is software-pipelined one supertile deep so the
DVE->gpsimd->DVE dependency (ranks -> masked -> extraction) never stalls
either in-order engine queue.
"""
import numpy as np

import concourse.bass as bass
import concourse.bacc as bacc
import concourse.mybir as mybir
from concourse.tile import TileContext
from concourse.bass_utils import run_bass_kernel_spmd

F32 = mybir.dt.float32
U32 = mybir.dt.uint32

T_FULL = 131072
E = 256
N_CORES = 8
T_CORE = T_FULL // N_CORES      # 16384
P = 128                         # tokens per tile (one per partition)
N_TILES = T_CORE // P           # 128
B = 8                           # tiles per supertile
N_SUPER = N_TILES // B          # 16
G = 8                           # expert groups
EG = E // G                     # experts per group
K_PACK = 16384.0                # carrier packing factor (2^14)

LAST_EXEC_NS = None
LAST_RESULTS = None

_OPS = None


def _get_ops():
    """Fused custom DVE ops:
      RTR_EQNEG:   out = (in0 == in1) ? -FLT_MAX : in0
      RTR_CARRIER: out = (in0 >= s0) ? (s1*in0 + in1) : 0
    """
    global _OPS
    if _OPS is None:
        from concourse.dve_ops import (DveOp, OPS, get_dve_sub_opcode,
                                       has_src1)
        from concourse.dve_spec import (Spec, Src0, Src1, C0, C1, MaxNeg,
                                        Zero, select, eq, lower)
        from concourse.dve_uop import DveOpSpec
        import concourse.dve_ops as dve_ops_mod

        def reg(name, spec):
            op = DveOp(name, spec, subdim=False, uops_sha={})
            OPS.append(op)
            dve_ops_mod.CUSTOM_DVE_SPECS[op.name] = op.spec
            dve_ops_mod._SUB_OPCODE_FOR_NAME[op.name] = (
                dve_ops_mod._CUSTOM_DVE_ROW_BASE + len(OPS) - 1)
            assert dve_ops_mod._SUB_OPCODE_FOR_NAME[op.name] < 0x20
            for ver in ("v3", "v4"):
                tmp = DveOpSpec(name=op.name,
                                opcode=get_dve_sub_opcode(op.name),
                                uops=lower(spec, ver=ver),
                                rd1_en=has_src1(spec))
                op.uops_sha[ver] = tmp.sha(ver)
            return op

        eqneg = reg("RTR_EQNEG", Spec(
            body=select(eq(Src0, Src1), MaxNeg, Src0),
            reference=lambda in0, in1, s0, s1, imm2: np.where(
                in0 == in1, np.float32(-3.4028234663852886e38), in0
            ).astype(np.float32)))

        carrier = reg("RTR_CARRIER", Spec(
            body=select(Src0 >= C0, C1 * Src0 + Src1, Zero),
            reference=lambda in0, in1, s0, s1, imm2: np.where(
                in0 >= s0,
                ((np.float32(s1) * in0).astype(np.float32)
                 + in1).astype(np.float32),
                np.float32(0.0)).astype(np.float32)))
        _OPS = (eqneg, carrier)
    return _OPS


def _build(nc: bass.Bass):
    x_d = nc.dram_tensor("logits", [T_CORE, E], F32, kind="ExternalInput")
    b8_d = nc.dram_tensor("bias8", [1, B * E], F32, kind="ExternalInput")
    idx_d = nc.dram_tensor("idx_out", [T_CORE, 8], U32, kind="ExternalOutput")
    w_d = nc.dram_tensor("w_out", [T_CORE, 8], F32, kind="ExternalOutput")

    AX = mybir.AxisListType.X
    OP = mybir.AluOpType
    ACTF = mybir.ActivationFunctionType
    eqneg, carrier = _get_ops()

    with TileContext(nc) as tc:
        with tc.tile_pool(name="const", bufs=1) as cpool, \
             tc.tile_pool(name="io", bufs=3) as iopool, \
             tc.tile_pool(name="wide", bufs=3) as wpool, \
             tc.tile_pool(name="small", bufs=3) as spool:

            biasb8 = cpool.tile([P, B * E], F32)
            nc.gpsimd.dma_start(out=biasb8[:],
                                in_=b8_d[:, :].to_broadcast((P, B * E)))

            # Per-supertile state carried across the two pipeline phases.
            state = [None] * (N_SUPER + 1)

            def phase1(sp_i):
                """load -> sigmoid -> bias add -> group top-2 -> ranks."""
                srow = sp_i * B * P
                Ls = iopool.tile([P, B * E], F32, tag="L")
                nc.sync.dma_start(
                    out=Ls[:],
                    in_=x_d[srow:srow + B * P, :].rearrange(
                        "(p x) e -> p (x e)", p=P))

                s = wpool.tile([P, B * E], F32, tag="s")
                nc.scalar.activation(s[:], Ls[:], ACTF.Sigmoid)

                s4c = wpool.tile([P, B * E], F32, tag="s4c")
                nc.gpsimd.tensor_tensor(s4c[:], s[:], biasb8[:], op=OP.add)
                s4c_g = s4c[:].rearrange("p (s n) -> p s n", n=EG)

                m1 = spool.tile([P, B * G], F32, tag="m1")
                nc.vector.tensor_reduce(m1[:], s4c_g, axis=AX, op=OP.max)
                t2 = iopool.tile([P, B * E], F32, tag="t2")
                nc.vector._custom_dve(
                    eqneg,
                    out=t2[:].rearrange("p (s n) -> p s n", n=EG),
                    in0=s4c_g,
                    in1=m1[:].unsqueeze(2).broadcast_to([P, B * G, EG]))
                m2 = spool.tile([P, B * G], F32, tag="m2")
                nc.vector.tensor_reduce(
                    m2[:], t2[:].rearrange("p (s n) -> p s n", n=EG),
                    axis=AX, op=OP.max)

                gss = spool.tile([P, B * G], F32, tag="gss")
                nc.vector.tensor_add(gss[:], m1[:], m2[:])
                gs3 = gss[:].rearrange("p (b g) -> p b g", b=B)
                gts = spool.tile([P, B * G * G], F32, tag="gts")
                nc.vector.tensor_tensor(
                    gts[:].rearrange("p (b i j) -> p b i j", b=B, i=G),
                    gs3.unsqueeze(2).broadcast_to([P, B, G, G]),
                    gs3.unsqueeze(3).broadcast_to([P, B, G, G]),
                    op=OP.is_gt)
                ranks = spool.tile([P, B * G], F32, tag="ranks")
                nc.vector.tensor_reduce(
                    ranks[:],
                    gts[:].rearrange("p (b i j) -> p b i j", b=B, i=G),
                    axis=AX, op=OP.add)
                return (srow, s4c, ranks)

            def phase2(st):
                """mask -> top-8 extract -> carrier -> weights -> store."""
                srow, s4c, ranks = st
                s4c_g = s4c[:].rearrange("p (s n) -> p s n", n=EG)

                mask01 = spool.tile([P, B * G], F32, tag="mask01")
                nc.vector.tensor_scalar(mask01[:], ranks[:], 4.0, None,
                                        op0=OP.is_lt)
                masked = wpool.tile([P, B * E], F32, tag="masked")
                nc.gpsimd.tensor_tensor(
                    masked[:].rearrange("p (s n) -> p s n", n=EG),
                    mask01[:].unsqueeze(2).broadcast_to([P, B * G, EG]),
                    s4c_g, op=OP.mult)

                v8s = spool.tile([P, B * 8], F32, tag="v8s")
                i8s = spool.tile([P, B * 8], U32, tag="i8s")
                z8s = spool.tile([P, B * 8], F32, tag="z8s")
                z = wpool.tile([P, B * E], F32, tag="z")
                for b in range(B):
                    mb = masked[:, b * E:(b + 1) * E]
                    v8 = v8s[:, b * 8:(b + 1) * 8]
                    nc.vector.max(out=v8, in_=mb)
                    nc.vector.max_index(i8s[:, b * 8:(b + 1) * 8], v8, mb)
                    zb = z[:, b * E:(b + 1) * E]
                    nc.vector._custom_dve(
                        carrier,
                        out=zb,
                        in0=mb,
                        in1=biasb8[:, b * E:(b + 1) * E],
                        s0=v8s[:, b * 8 + 7:b * 8 + 8],
                        s1=K_PACK)
                    nc.vector.max(out=z8s[:, b * 8:(b + 1) * 8], in_=zb)

                # s at selected = (1+K)*v8 - z8 (z8 is in v8 order)
                w8 = spool.tile([P, B * 8], F32, tag="w8")
                nc.vector.scalar_tensor_tensor(
                    w8[:], v8s[:], 1.0 + K_PACK, z8s[:],
                    op0=OP.mult, op1=OP.subtract)
                dens = spool.tile([P, B], F32, tag="dens")
                nc.vector.tensor_reduce(
                    dens[:], w8[:].rearrange("p (b k) -> p b k", b=B),
                    axis=AX, op=OP.add)
                rdens = spool.tile([P, B], F32, tag="rdens")
                nc.vector.reciprocal(rdens[:], dens[:])
                wouts = spool.tile([P, B * 8], F32, tag="wouts")
                nc.vector.scalar_tensor_tensor(
                    wouts[:].rearrange("p (b k) -> p b k", b=B),
                    w8[:].rearrange("p (b k) -> p b k", b=B), 2.5,
                    rdens[:].unsqueeze(2).broadcast_to([P, B, 8]),
                    op0=OP.mult, op1=OP.mult)

                nc.sync.dma_start(
                    out=idx_d[srow:srow + B * P, :].rearrange(
                        "(p x) e -> p (x e)", p=P),
                    in_=i8s[:])
                nc.sync.dma_start(
                    out=w_d[srow:srow + B * P, :].rearrange(
                        "(p x) e -> p (x e)", p=P),
                    in_=wouts[:])

            # Software pipeline: phase1(sp) runs ahead of phase2(sp-1).
            for sp_i in range(N_SUPER + 1):
                if sp_i < N_SUPER:
                    state[sp_i] = phase1(sp_i)
                if sp_i > 0:
                    phase2(state[sp_i - 1])
                    state[sp_i - 1] = None
    return nc


_COMPILED_NC = None


def _get_nc():
    global _COMPILED_NC
    if _COMPILED_NC is None:
        nc = bacc.Bacc(None, target_bir_lowering=False, debug=False)
        _build(nc)
        nc.finalize()
        _COMPILED_NC = nc
    return _COMPILED_NC


def kernel(router_logits: np.ndarray, correction_bias: np.ndarray,
           trace: bool = False):
    global LAST_EXEC_NS, LAST_RESULTS
    x = np.ascontiguousarray(np.asarray(router_logits), dtype=np.float32)
    b = np.ascontiguousarray(np.asarray(correction_bias),
                             dtype=np.float32).reshape(E)
    assert x.shape == (T_FULL, E), x.shape
    b8 = np.tile(b, B).reshape(1, B * E)

    nc = _get_nc()
    in_maps = [{"logits": x[c * T_CORE:(c + 1) * T_CORE], "bias8": b8}
               for c in range(N_CORES)]
    res = run_bass_kernel_spmd(nc, in_maps, core_ids=list(range(N_CORES)),
                               trace=trace)
    LAST_EXEC_NS = res.exec_time_ns
    LAST_RESULTS = res

    idx = np.concatenate([r["idx_out"] for r in res.results], axis=0)
    w = np.concatenate([r["w_out"] for r in res.results], axis=0)
    return idx.view(np.int32), w.astype(np.float32, copy=False)


# revision 5
# speedup vs baseline: 1.7708x; 1.0106x over previous
"""DeepseekV3 top-k router kernel for Trainium2 (Bass/Tile), 8-core SPMD.

Reference computation (per token, 256 experts):
  s    = sigmoid(logits)           ACT-engine Sigmoid table (~1e-6 max abs err)
  s4c  = s + correction_bias       gpsimd add
  group scores = sum of top-2 of s4c within each of 8 groups of 32
  top-4 groups -> mask -> masked s4c
  top-8 of masked s4c -> (indices, values)   [DVE Max8/FindIndex8]
  weights = s at those indices, normalized to sum 2.5

Weight extraction avoids a second FindIndex8 + index matching via a packed
"carrier": z = K*s4c + bias at the 8 selected positions (0 elsewhere), with
K = 2^14.  Max8(z) returns the selected entries in (approximately) the same
order as Max8(masked); then s_at_selected = (1+K)*v8 - z8 elementwise.  The
packing costs ~1.5e-3 relative error on the weights (fp32 low-bit truncation
of the bias payload + rare near-tie order swaps), well under the 2e-2 gate;
indices are unaffected.

Sharding: data-parallel on the token dim across 8 cores (16384 tokens each);
the 256-entry bias is replicated (host pre-tiles it 8x to [1, 2048]).
Layout: one token per SBUF partition, its 256 expert scores along the free
dim; 128 tokens per tile, 8 tiles per supertile [128, 2048] so the wide
stages run as single instructions.

Engine split: ACT does the sigmoid; gpsimd does the bias add and the
group-mask application; DVE does the group maxes (second max via a fused
custom eqneg op), rank compare, Max8/FindIndex8, and the carrier op.  The
per-supertile loop is software-pipelined one supertile deep so the
DVE->gpsimd->DVE dependency (ranks -> masked -> extraction) never stalls
either in-order engine queue.
"""
import numpy as np

import concourse.bass as bass
import concourse.bacc as bacc
import concourse.mybir as mybir
from concourse.tile import TileContext
from concourse.bass_utils import run_bass_kernel_spmd

F32 = mybir.dt.float32
U32 = mybir.dt.uint32

T_FULL = 131072
E = 256
N_CORES = 8
T_CORE = T_FULL // N_CORES      # 16384
P = 128                         # tokens per tile (one per partition)
N_TILES = T_CORE // P           # 128
B = 8                           # tiles per supertile
N_SUPER = N_TILES // B          # 16
G = 8                           # expert groups
EG = E // G                     # experts per group
K_PACK = 16384.0                # carrier packing factor (2^14)

LAST_EXEC_NS = None
LAST_RESULTS = None

_OPS = None


def _get_ops():
    """Fused custom DVE ops:
      RTR_EQNEG:   out = (in0 == in1) ? -FLT_MAX : in0
      RTR_CARRIER: out = (in0 >= s0) ? (s1*in0 + in1) : 0
    """
    global _OPS
    if _OPS is None:
        from concourse.dve_ops import (DveOp, OPS, get_dve_sub_opcode,
                                       has_src1)
        from concourse.dve_spec import (Spec, Src0, Src1, C0, C1, MaxNeg,
                                        Zero, select, eq, lower)
        from concourse.dve_uop import DveOpSpec
        import concourse.dve_ops as dve_ops_mod

        def reg(name, spec):
            op = DveOp(name, spec, subdim=False, uops_sha={})
            OPS.append(op)
            dve_ops_mod.CUSTOM_DVE_SPECS[op.name] = op.spec
            dve_ops_mod._SUB_OPCODE_FOR_NAME[op.name] = (
                dve_ops_mod._CUSTOM_DVE_ROW_BASE + len(OPS) - 1)
            assert dve_ops_mod._SUB_OPCODE_FOR_NAME[op.name] < 0x20
            for ver in ("v3", "v4"):
                tmp = DveOpSpec(name=op.name,
                                opcode=get_dve_sub_opcode(op.name),
                                uops=lower(spec, ver=ver),
                                rd1_en=has_src1(spec))
                op.uops_sha[ver] = tmp.sha(ver)
            return op

        eqneg = reg("RTR_EQNEG", Spec(
            body=select(eq(Src0, Src1), MaxNeg, Src0),
            reference=lambda in0, in1, s0, s1, imm2: np.where(
                in0 == in1, np.float32(-3.4028234663852886e38), in0
            ).astype(np.float32)))

        carrier = reg("RTR_MADD", Spec(
            body=C1 * Src0 + Src1,
            reference=lambda in0, in1, s0, s1, imm2: (
                (np.float32(s1) * in0).astype(np.float32)
                + in1).astype(np.float32)))
        _OPS = (eqneg, carrier)
    return _OPS


def _build(nc: bass.Bass):
    x_d = nc.dram_tensor("logits", [T_CORE, E], F32, kind="ExternalInput")
    b8_d = nc.dram_tensor("bias8", [1, B * E], F32, kind="ExternalInput")
    idx_d = nc.dram_tensor("idx_out", [T_CORE, 8], U32, kind="ExternalOutput")
    w_d = nc.dram_tensor("w_out", [T_CORE, 8], F32, kind="ExternalOutput")

    AX = mybir.AxisListType.X
    OP = mybir.AluOpType
    ACTF = mybir.ActivationFunctionType
    eqneg, carrier = _get_ops()

    with TileContext(nc) as tc:
        with tc.tile_pool(name="const", bufs=1) as cpool, \
             tc.tile_pool(name="io", bufs=3) as iopool, \
             tc.tile_pool(name="wide", bufs=3) as wpool, \
             tc.tile_pool(name="small", bufs=3) as spool:

            biasb8 = cpool.tile([P, B * E], F32)
            nc.gpsimd.dma_start(out=biasb8[:],
                                in_=b8_d[:, :].to_broadcast((P, B * E)))

            # Per-supertile state carried across the two pipeline phases.
            state = [None] * (N_SUPER + 1)

            def phase1(sp_i):
                """load -> sigmoid -> bias add -> group top-2 -> ranks."""
                srow = sp_i * B * P
                Ls = iopool.tile([P, B * E], F32, tag="L")
                nc.sync.dma_start(
                    out=Ls[:],
                    in_=x_d[srow:srow + B * P, :].rearrange(
                        "(p x) e -> p (x e)", p=P))

                s = wpool.tile([P, B * E], F32, tag="s")
                nc.scalar.activation(s[:], Ls[:], ACTF.Sigmoid)

                s4c = wpool.tile([P, B * E], F32, tag="s4c")
                nc.gpsimd.tensor_tensor(s4c[:], s[:], biasb8[:], op=OP.add)
                s4c_g = s4c[:].rearrange("p (s n) -> p s n", n=EG)

                m1 = spool.tile([P, B * G], F32, tag="m1")
                nc.vector.tensor_reduce(m1[:], s4c_g, axis=AX, op=OP.max)
                t2 = iopool.tile([P, B * E], F32, tag="t2")
                nc.vector._custom_dve(
                    eqneg,
                    out=t2[:].rearrange("p (s n) -> p s n", n=EG),
                    in0=s4c_g,
                    in1=m1[:].unsqueeze(2).broadcast_to([P, B * G, EG]))
                m2 = spool.tile([P, B * G], F32, tag="m2")
                nc.vector.tensor_reduce(
                    m2[:], t2[:].rearrange("p (s n) -> p s n", n=EG),
                    axis=AX, op=OP.max)

                gss = spool.tile([P, B * G], F32, tag="gss")
                nc.vector.tensor_add(gss[:], m1[:], m2[:])
                gs3 = gss[:].rearrange("p (b g) -> p b g", b=B)
                gts = spool.tile([P, B * G * G], F32, tag="gts")
                nc.vector.tensor_tensor(
                    gts[:].rearrange("p (b i j) -> p b i j", b=B, i=G),
                    gs3.unsqueeze(2).broadcast_to([P, B, G, G]),
                    gs3.unsqueeze(3).broadcast_to([P, B, G, G]),
                    op=OP.is_gt)
                ranks = spool.tile([P, B * G], F32, tag="ranks")
                nc.vector.tensor_reduce(
                    ranks[:],
                    gts[:].rearrange("p (b i j) -> p b i j", b=B, i=G),
                    axis=AX, op=OP.add)
                return (srow, s4c, ranks)

            def phase2(st):
                """mask -> top-8 extract -> carrier -> weights -> store."""
                srow, s4c, ranks = st
                s4c_g = s4c[:].rearrange("p (s n) -> p s n", n=EG)

                mask01 = spool.tile([P, B * G], F32, tag="mask01")
                nc.vector.tensor_scalar(mask01[:], ranks[:], 4.0, None,
                                        op0=OP.is_lt)
                masked = wpool.tile([P, B * E], F32, tag="masked")
                nc.gpsimd.tensor_tensor(
                    masked[:].rearrange("p (s n) -> p s n", n=EG),
                    mask01[:].unsqueeze(2).broadcast_to([P, B * G, EG]),
                    s4c_g, op=OP.mult)

                # carrier: z = K*masked + bias everywhere (no threshold --
                # non-top-group entries land at ~bias, never in the top-8)
                z = wpool.tile([P, B * E], F32, tag="z")
                nc.vector._custom_dve(
                    carrier, out=z[:], in0=masked[:], in1=biasb8[:],
                    s1=K_PACK)

                v8s = spool.tile([P, B * 8], F32, tag="v8s")
                i8s = spool.tile([P, B * 8], U32, tag="i8s")
                z8s = spool.tile([P, B * 8], F32, tag="z8s")
                for b in range(B):
                    mb = masked[:, b * E:(b + 1) * E]
                    v8 = v8s[:, b * 8:(b + 1) * 8]
                    nc.vector.max(out=v8, in_=mb)
                    nc.vector.max_index(i8s[:, b * 8:(b + 1) * 8], v8, mb)
                    nc.vector.max(out=z8s[:, b * 8:(b + 1) * 8],
                                  in_=z[:, b * E:(b + 1) * E])

                # s at selected = (1+K)*v8 - z8 (z8 is in v8 order)
                w8 = spool.tile([P, B * 8], F32, tag="w8")
                nc.vector.scalar_tensor_tensor(
                    w8[:], v8s[:], 1.0 + K_PACK, z8s[:],
                    op0=OP.mult, op1=OP.subtract)
                dens = spool.tile([P, B], F32, tag="dens")
                nc.vector.tensor_reduce(
                    dens[:], w8[:].rearrange("p (b k) -> p b k", b=B),
                    axis=AX, op=OP.add)
                rdens = spool.tile([P, B], F32, tag="rdens")
                nc.vector.reciprocal(rdens[:], dens[:])
                nc.vector.tensor_scalar(rdens[:], rdens[:], 2.5, None,
                                        op0=OP.mult)
                wouts = spool.tile([P, B * 8], F32, tag="wouts")
                nc.vector.tensor_tensor(
                    wouts[:].rearrange("p (b k) -> p b k", b=B),
                    w8[:].rearrange("p (b k) -> p b k", b=B),
                    rdens[:].unsqueeze(2).broadcast_to([P, B, 8]),
                    op=OP.mult)

                nc.sync.dma_start(
                    out=idx_d[srow:srow + B * P, :].rearrange(
                        "(p x) e -> p (x e)", p=P),
                    in_=i8s[:])
                nc.sync.dma_start(
                    out=w_d[srow:srow + B * P, :].rearrange(
                        "(p x) e -> p (x e)", p=P),
                    in_=wouts[:])

            # Software pipeline: phase1(sp) runs ahead of phase2(sp-1).
            for sp_i in range(N_SUPER + 1):
                if sp_i < N_SUPER:
                    state[sp_i] = phase1(sp_i)
                if sp_i > 0:
                    phase2(state[sp_i - 1])
                    state[sp_i - 1] = None
    return nc


_COMPILED_NC = None


def _get_nc():
    global _COMPILED_NC
    if _COMPILED_NC is None:
        nc = bacc.Bacc(None, target_bir_lowering=False, debug=False)
        _build(nc)
        nc.finalize()
        _COMPILED_NC = nc
    return _COMPILED_NC


def kernel(router_logits: np.ndarray, correction_bias: np.ndarray,
           trace: bool = False):
    global LAST_EXEC_NS, LAST_RESULTS
    x = np.ascontiguousarray(np.asarray(router_logits), dtype=np.float32)
    b = np.ascontiguousarray(np.asarray(correction_bias),
                             dtype=np.float32).reshape(E)
    assert x.shape == (T_FULL, E), x.shape
    b8 = np.tile(b, B).reshape(1, B * E)

    nc = _get_nc()
    in_maps = [{"logits": x[c * T_CORE:(c + 1) * T_CORE], "bias8": b8}
               for c in range(N_CORES)]
    res = run_bass_kernel_spmd(nc, in_maps, core_ids=list(range(N_CORES)),
                               trace=trace)
    LAST_EXEC_NS = res.exec_time_ns
    LAST_RESULTS = res

    idx = np.concatenate([r["idx_out"] for r in res.results], axis=0)
    w = np.concatenate([r["w_out"] for r in res.results], axis=0)
    return idx.view(np.int32), w.astype(np.float32, copy=False)


# revision 10
# speedup vs baseline: 1.8771x; 1.0600x over previous
"""DeepseekV3 top-k router kernel for Trainium2 (Bass/Tile), 8-core SPMD.

Reference computation (per token, 256 experts):
  s    = sigmoid(logits)           ACT-engine Sigmoid table (~1e-6 max abs err)
  s4c  = s + correction_bias       gpsimd add
  group scores = sum of top-2 of s4c within each of 8 groups of 32
  top-4 groups -> mask -> masked s4c
  top-8 of masked s4c -> (indices, values)   [DVE Max8/FindIndex8]
  weights = s at those indices, normalized to sum 2.5

Weight extraction avoids a second FindIndex8 + index matching via a packed
"carrier": z = K*s4c + bias at the 8 selected positions (0 elsewhere), with
K = 2^14.  Max8(z) returns the selected entries in (approximately) the same
order as Max8(masked); then s_at_selected = (1+K)*v8 - z8 elementwise.  The
packing costs ~1.5e-3 relative error on the weights (fp32 low-bit truncation
of the bias payload + rare near-tie order swaps), well under the 2e-2 gate;
indices are unaffected.

Sharding: data-parallel on the token dim across 8 cores (16384 tokens each);
the 256-entry bias is replicated (host pre-tiles it 8x to [1, 2048]).
Layout: one token per SBUF partition, its 256 expert scores along the free
dim; 128 tokens per tile, 8 tiles per supertile [128, 2048] so the wide
stages run as single instructions.

Engine split: ACT does the sigmoid; gpsimd does the bias add and the
group-mask application; DVE does the group maxes (second max via a fused
custom eqneg op), rank compare, Max8/FindIndex8, and the carrier op.  The
per-supertile loop is software-pipelined one supertile deep so the
DVE->gpsimd->DVE dependency (ranks -> masked -> extraction) never stalls
either in-order engine queue.
"""
import numpy as np

import concourse.bass as bass
import concourse.bacc as bacc
import concourse.mybir as mybir
from concourse.tile import TileContext
from concourse.bass_utils import run_bass_kernel_spmd

F32 = mybir.dt.float32
U32 = mybir.dt.uint32

T_FULL = 131072
E = 256
N_CORES = 8
T_CORE = T_FULL // N_CORES      # 16384
P = 128                         # tokens per tile (one per partition)
N_TILES = T_CORE // P           # 128
B = 8                           # tiles per supertile
N_SUPER = N_TILES // B          # 16
G = 8                           # expert groups
EG = E // G                     # experts per group
K_PACK = 16384.0                # carrier packing factor (2^14)

LAST_EXEC_NS = None
LAST_RESULTS = None

_OPS = None


def _get_ops():
    """Fused custom DVE ops:
      RTR_EQNEG:   out = (in0 == in1) ? -FLT_MAX : in0
      RTR_CARRIER: out = (in0 >= s0) ? (s1*in0 + in1) : 0
    """
    global _OPS
    if _OPS is None:
        from concourse.dve_ops import (DveOp, OPS, get_dve_sub_opcode,
                                       has_src1)
        from concourse.dve_spec import (Spec, Src0, Src1, C0, C1, MaxNeg,
                                        Zero, select, eq, lower)
        from concourse.dve_uop import DveOpSpec
        import concourse.dve_ops as dve_ops_mod

        def reg(name, spec):
            op = DveOp(name, spec, subdim=False, uops_sha={})
            OPS.append(op)
            dve_ops_mod.CUSTOM_DVE_SPECS[op.name] = op.spec
            dve_ops_mod._SUB_OPCODE_FOR_NAME[op.name] = (
                dve_ops_mod._CUSTOM_DVE_ROW_BASE + len(OPS) - 1)
            assert dve_ops_mod._SUB_OPCODE_FOR_NAME[op.name] < 0x20
            for ver in ("v3", "v4"):
                tmp = DveOpSpec(name=op.name,
                                opcode=get_dve_sub_opcode(op.name),
                                uops=lower(spec, ver=ver),
                                rd1_en=has_src1(spec))
                op.uops_sha[ver] = tmp.sha(ver)
            return op

        eqneg = reg("RTR_EQNEG", Spec(
            body=select(eq(Src0, Src1), MaxNeg, Src0),
            reference=lambda in0, in1, s0, s1, imm2: np.where(
                in0 == in1, np.float32(-3.4028234663852886e38), in0
            ).astype(np.float32)))

        _OPS = (eqneg,)
    return _OPS


def _build(nc: bass.Bass):
    x_d = nc.dram_tensor("logits", [T_CORE, E], F32, kind="ExternalInput")
    b8_d = nc.dram_tensor("bias8", [1, B * E], F32, kind="ExternalInput")
    bok8_d = nc.dram_tensor("biasok8", [1, B * E], F32, kind="ExternalInput")
    idx_d = nc.dram_tensor("idx_out", [T_CORE, 8], U32, kind="ExternalOutput")
    w_d = nc.dram_tensor("w_out", [T_CORE, 8], F32, kind="ExternalOutput")

    AX = mybir.AxisListType.X
    OP = mybir.AluOpType
    ACTF = mybir.ActivationFunctionType
    (eqneg,) = _get_ops()

    with TileContext(nc) as tc:
        with tc.tile_pool(name="const", bufs=1) as cpool, \
             tc.tile_pool(name="io", bufs=3) as iopool, \
             tc.tile_pool(name="wide", bufs=3) as wpool, \
             tc.tile_pool(name="small", bufs=3) as spool:

            biasb8 = cpool.tile([P, B * E], F32)
            nc.gpsimd.dma_start(out=biasb8[:],
                                in_=b8_d[:, :].to_broadcast((P, B * E)))
            biasok8 = cpool.tile([P, B * E], F32)
            nc.gpsimd.dma_start(out=biasok8[:],
                                in_=bok8_d[:, :].to_broadcast((P, B * E)))

            # Per-supertile state carried across the two pipeline phases.
            state = [None] * (N_SUPER + 1)

            def phase1(sp_i):
                """load -> sigmoid -> bias add -> group top-2 -> ranks."""
                srow = sp_i * B * P
                Ls = iopool.tile([P, B * E], F32, tag="L")
                nc.sync.dma_start(
                    out=Ls[:],
                    in_=x_d[srow:srow + B * P, :].rearrange(
                        "(p x) e -> p (x e)", p=P))

                s = wpool.tile([P, B * E], F32, tag="s")
                nc.scalar.activation(s[:], Ls[:], ACTF.Sigmoid)

                s4c = wpool.tile([P, B * E], F32, tag="s4c")
                nc.gpsimd.tensor_tensor(s4c[:], s[:], biasb8[:], op=OP.add)
                s4c_g = s4c[:].rearrange("p (s n) -> p s n", n=EG)

                m1 = spool.tile([P, B * G], F32, tag="m1")
                nc.vector.tensor_reduce(m1[:], s4c_g, axis=AX, op=OP.max)
                t2 = iopool.tile([P, B * E], F32, tag="t2")
                nc.vector._custom_dve(
                    eqneg,
                    out=t2[:].rearrange("p (s n) -> p s n", n=EG),
                    in0=s4c_g,
                    in1=m1[:].unsqueeze(2).broadcast_to([P, B * G, EG]))
                m2 = spool.tile([P, B * G], F32, tag="m2")
                nc.vector.tensor_reduce(
                    m2[:], t2[:].rearrange("p (s n) -> p s n", n=EG),
                    axis=AX, op=OP.max)

                gss = spool.tile([P, B * G], F32, tag="gss")
                nc.vector.tensor_add(gss[:], m1[:], m2[:])
                gs3 = gss[:].rearrange("p (b g) -> p b g", b=B)
                gts = spool.tile([P, B * G * G], F32, tag="gts")
                nc.vector.tensor_tensor(
                    gts[:].rearrange("p (b i j) -> p b i j", b=B, i=G),
                    gs3.unsqueeze(2).broadcast_to([P, B, G, G]),
                    gs3.unsqueeze(3).broadcast_to([P, B, G, G]),
                    op=OP.is_gt)
                ranks = spool.tile([P, B * G], F32, tag="ranks")
                nc.vector.tensor_reduce(
                    ranks[:],
                    gts[:].rearrange("p (b i j) -> p b i j", b=B, i=G),
                    axis=AX, op=OP.add)
                return (srow, s4c, ranks)

            def phase2(st):
                """mask -> top-8 extract -> carrier -> weights -> store."""
                srow, s4c, ranks = st
                s4c_g = s4c[:].rearrange("p (s n) -> p s n", n=EG)

                mask01 = spool.tile([P, B * G], F32, tag="mask01")
                nc.vector.tensor_scalar(mask01[:], ranks[:], 4.0, None,
                                        op0=OP.is_lt)
                masked = wpool.tile([P, B * E], F32, tag="masked")
                nc.gpsimd.tensor_tensor(
                    masked[:].rearrange("p (s n) -> p s n", n=EG),
                    mask01[:].unsqueeze(2).broadcast_to([P, B * G, EG]),
                    s4c_g, op=OP.mult)

                # carrier: z = masked + bias/K everywhere (no threshold --
                # non-top-group entries land at ~bias/K, never in the top-8;
                # the K scale cancels in the normalization)
                z = wpool.tile([P, B * E], F32, tag="z")
                nc.gpsimd.tensor_tensor(z[:], masked[:], biasok8[:],
                                        op=OP.add)

                v8s = spool.tile([P, B * 8], F32, tag="v8s")
                i8s = spool.tile([P, B * 8], U32, tag="i8s")
                z8s = spool.tile([P, B * 8], F32, tag="z8s")
                for b in range(B):
                    nc.vector.max(out=v8s[:, b * 8:(b + 1) * 8],
                                  in_=masked[:, b * E:(b + 1) * E])
                for b in range(B):
                    nc.vector.max(out=z8s[:, b * 8:(b + 1) * 8],
                                  in_=z[:, b * E:(b + 1) * E])
                for b in range(B):
                    nc.vector.max_index(i8s[:, b * 8:(b + 1) * 8],
                                        v8s[:, b * 8:(b + 1) * 8],
                                        masked[:, b * E:(b + 1) * E])

                # s at selected (scaled by 1/K): t = (1+1/K)*v8 - z8
                w8 = spool.tile([P, B * 8], F32, tag="w8")
                nc.vector.scalar_tensor_tensor(
                    w8[:], v8s[:], 1.0 + 1.0 / K_PACK, z8s[:],
                    op0=OP.mult, op1=OP.subtract)
                dens = spool.tile([P, B], F32, tag="dens")
                nc.vector.tensor_reduce(
                    dens[:], w8[:].rearrange("p (b k) -> p b k", b=B),
                    axis=AX, op=OP.add)
                rdens = spool.tile([P, B], F32, tag="rdens")
                nc.vector.reciprocal(rdens[:], dens[:])
                nc.vector.tensor_scalar(rdens[:], rdens[:], 2.5, None,
                                        op0=OP.mult)
                rd64 = spool.tile([P, B * 8], F32, tag="rd64")
                nc.vector.tensor_copy(
                    rd64[:].rearrange("p (b k) -> p b k", b=B),
                    rdens[:].unsqueeze(2).broadcast_to([P, B, 8]))
                wouts = spool.tile([P, B * 8], F32, tag="wouts")
                nc.vector.tensor_tensor(wouts[:], w8[:], rd64[:], op=OP.mult)

                nc.sync.dma_start(
                    out=idx_d[srow:srow + B * P, :].rearrange(
                        "(p x) e -> p (x e)", p=P),
                    in_=i8s[:])
                nc.sync.dma_start(
                    out=w_d[srow:srow + B * P, :].rearrange(
                        "(p x) e -> p (x e)", p=P),
                    in_=wouts[:])

            # Software pipeline: phase1(sp) runs ahead of phase2(sp-1).
            for sp_i in range(N_SUPER + 1):
                if sp_i < N_SUPER:
                    state[sp_i] = phase1(sp_i)
                if sp_i > 0:
                    phase2(state[sp_i - 1])
                    state[sp_i - 1] = None
    return nc


_COMPILED_NC = None


def _get_nc():
    global _COMPILED_NC
    if _COMPILED_NC is None:
        nc = bacc.Bacc(None, target_bir_lowering=False, debug=False)
        _build(nc)
        nc.finalize()
        _COMPILED_NC = nc
    return _COMPILED_NC


def kernel(router_logits: np.ndarray, correction_bias: np.ndarray,
           trace: bool = False):
    global LAST_EXEC_NS, LAST_RESULTS
    x = np.ascontiguousarray(np.asarray(router_logits), dtype=np.float32)
    b = np.ascontiguousarray(np.asarray(correction_bias),
                             dtype=np.float32).reshape(E)
    assert x.shape == (T_FULL, E), x.shape
    b8 = np.tile(b, B).reshape(1, B * E)
    bok8 = (b8 / np.float32(K_PACK)).astype(np.float32)

    nc = _get_nc()
    in_maps = [{"logits": x[c * T_CORE:(c + 1) * T_CORE], "bias8": b8,
                "biasok8": bok8}
               for c in range(N_CORES)]
    res = run_bass_kernel_spmd(nc, in_maps, core_ids=list(range(N_CORES)),
                               trace=trace)
    LAST_EXEC_NS = res.exec_time_ns
    LAST_RESULTS = res

    idx = np.concatenate([r["idx_out"] for r in res.results], axis=0)
    w = np.concatenate([r["w_out"] for r in res.results], axis=0)
    return idx.view(np.int32), w.astype(np.float32, copy=False)
